# revision 55
# baseline (speedup 1.0000x reference)
"""TRN2 Bass kernel: differentiable palette quantization (soft VQ).

  weights = softmax_k( -|x - p_k|^2 / T );  out = sum_k weights_k p_k

Data-parallel over 8 NeuronCores (4 images each). Pipeline per core:

  mm1 (PE, 4-way row-tiled):  logits = 2/T * x.p  ->  PSUM f32
  exp (ACT table / DVE Schraudolph split):        ->  e f16 SBUF
  mm2 (PE, 4-way col-tiled, M=32):                ->  PSUM f32
  copy (DVE) -> y f16; denominator gather (DMA) -> sd
  recip via Ln/-Exp (ACT); rb row-broadcast (DMA); o = y*rb (DVE 2x)

Granule = 2 blocks = 8192 px; 32 granules/core; 8 granules per image.
PSUM: 6-bank p1 ring arena + 2x 1-bank p2.

Row layout within each 32-row output group (so every DMA partition
pattern is a single stride with offset % stride == 0): output
s = 8u is the denominator, s = 8u + 1 + c the channels c<3, rest
zero. sd/r row = 16*gi + 4j + u. rb replicates r to all 8 slots via
one stride-0-source DMA; out DMAs the full [128,512] tile.

Self-contained: includes the walrus sync-wait-limit workaround (this
toolchain allows ONE semaphore wait per instruction) and the Tile exit
drain patch.
"""

import sys

sys.path.insert(0, "/opt/trn_rl_repo")

import math

import numpy as np

import concourse.bass as bass
import concourse.tile as tile
from concourse import mybir
from concourse.tile import ScopedClock

B, H, W, C, K = 32, 256, 256, 3, 32
NPIX = H * W
N_CORES = 8
IMGS = B // N_CORES      # 4 images per core
NGRAN = 32               # granules per core (2 blocks each)
GPI = 8                  # granules per image
F32 = mybir.dt.float32
F16 = mybir.dt.float16
I16 = mybir.dt.int16

SCH_S = float(1024.0 / math.log(2.0))   # Schraudolph scale
SCH_SIGMA = 0.06
SCH_M = float(1024.0 * (15.0 - SCH_SIGMA))

# Granules routed to the DVE fast-exp instead of the ACT exp table.
DVE_GRAN = {g for g in range(32) if g % 5 == 2}



# ---------------------------------------------------------------------------
# Toolchain workarounds
# ---------------------------------------------------------------------------

_MAX_WAITS = 1


def _split_excess_waits(nc):
    """This walrus build rejects >1 sync wait per instruction. Move the
    excess onto same-engine NOPs inserted immediately before."""
    for f in nc.m.functions:
        for bb in f.blocks:
            insts = bb.instructions
            if not any(
                i.sync_info is not None and len(i.sync_info.on_wait) > _MAX_WAITS
                for i in insts
            ):
                continue
            new = []
            for inst in insts:
                si = inst.sync_info
                waits = list(si.on_wait) if si is not None else []
                if len(waits) > _MAX_WAITS:
                    extra, keep = waits[:-_MAX_WAITS], waits[-_MAX_WAITS:]
                    for i in range(0, len(extra), _MAX_WAITS):
                        new.append(
                            mybir.InstNoOp(
                                name=nc.get_next_instruction_name(),
                                engine=inst.engine,
                                bass_nofuse=True,
                                sync_info=mybir.SyncInfo(
                                    on_wait=extra[i : i + _MAX_WAITS], on_update=[]
                                ),
                            )
                        )
                    inst.sync_info = mybir.SyncInfo(
                        on_wait=keep, on_update=list(si.on_update)
                    )
                new.append(inst)
            bb.instructions = new


def _patched_drain_and_barrier(self, tick_clock, wait_clock):
    """Tile's exit drain carries one wait per active proc; spread them
    across single-wait NOPs (same walrus limit as above)."""
    nc = self.nc
    probe = nc.sync.nop(nofuse=True, hint="drain_waits")
    wait_clock.add_sem_waits(probe.ins, ScopedClock({None: tick_clock.global_clock}))
    si = probe.ins.sync_info
    waits = list(si.on_wait) if si is not None else []
    updates = list(si.on_update) if si is not None else []
    if len(waits) > 1:
        probe.ins.sync_info = mybir.SyncInfo(on_wait=waits[:1], on_update=updates)
        for i, w in enumerate(waits[1:]):
            extra = nc.sync.nop(nofuse=True, hint=f"drain_waits_{i}")
            extra.ins.sync_info = mybir.SyncInfo(on_wait=[w], on_update=[])
    nc.sync.drain()
    nc.all_engine_barrier()
    assert self.sems is not None
    popped = nc._tile_sem_poison_stack.pop()
    assert popped is self._sem_poison
    nc.clear_and_free_semaphores(list(self.sems.allocated().values()))
    nc.all_engine_barrier()


tile.TileContext._drain_and_barrier = _patched_drain_and_barrier

# ---------------------------------------------------------------------------
# Program builder
# ---------------------------------------------------------------------------


def _build_program():
    nc = bass.Bass()
    NP = NGRAN // 2  # 16 granule-pairs
    x = nc.dram_tensor("x", [NP, 128, 1024], F16, kind="ExternalInput")
    w1 = nc.dram_tensor("w1", [128, 128 * IMGS], F16, kind="ExternalInput")
    w2 = nc.dram_tensor("w2", [128, 32 * IMGS], F16, kind="ExternalInput")
    bias = nc.dram_tensor("bias", [128, IMGS], F32, kind="ExternalInput")
    vbias = nc.dram_tensor("vbias", [128, IMGS], F32, kind="ExternalInput")
    out = nc.dram_tensor("out", [NP, 128, 1024], F16, kind="ExternalOutput")

    with tile.TileContext(nc) as tc:
        with (
            tc.tile_pool(name="singles", bufs=1) as singles,
            tc.tile_pool(name="xt", bufs=4) as xt_pool,
            tc.tile_pool(name="e", bufs=4) as e_pool,
            tc.tile_pool(name="arena", bufs=1, space="PSUM") as arena_pool,
            tc.tile_pool(name="y", bufs=8) as y_pool,
            tc.tile_pool(name="o", bufs=3) as o_pool,
        ):
            w1sb = singles.tile([128, 128 * IMGS], F16)
            w2sb = singles.tile([128, 32 * IMGS], F16)
            bsb = singles.tile([128, IMGS], F32)
            vsb = singles.tile([128, IMGS], F32)
            # sd/r layout: row = 64*(img%2) + 16*q + t  (q = pair-in-image,
            # t = 4j+u), col = 1024*(img//2) + 512*(g%2) + n.
            sd = singles.tile([128, 512 * IMGS], F16)
            r = singles.tile([128, 512 * IMGS], F16)
            rbs = []
            for i in range(3):
                rbt = singles.tile([128, 1024], F16, name=f"rbt{i}")
                nc.vector.memset(rbt[:], 0.0)
                rbs.append(rbt)
            # init loads off the sync queue so the first xt DMAs go first;
            # scratch exp preloads the ACT exp/ln table during the DMAs.
            scratch = singles.tile([128, 1], F16)
            nc.gpsimd.dma_start(out=w1sb[:], in_=w1[:])
            nc.scalar.dma_start(out=w2sb[:], in_=w2[:])
            nc.gpsimd.dma_start(out=bsb[:], in_=bias[:])
            nc.gpsimd.dma_start(out=vsb[:], in_=vbias[:])
            nc.vector.memset(scratch[:], 1.0)
            nc.scalar.activation(
                out=scratch[:], in_=scratch[:],
                func=mybir.ActivationFunctionType.Exp,
            )

            # All 8 PSUM banks: two pair-regions of [128,2048] (4 block-
            # slots). mm1(g) fills pair g%2; exp(g) reads it as one N=2048;
            # mm2(g) then reuses the first bank of the pair as its p2.
            arena = arena_pool.tile([128, 4096], F32)

            def load_x(p):
                xt = xt_pool.tile([128, 1024], F16, name="xt")
                with tc.high_priority(offset=40):
                    nc.sync.dma_start(out=xt[:], in_=x[p])
                return xt

            def mm1_one(g, xt, i):
                img = g // GPI
                off = 2048 * (g % 2) + 512 * i
                nc.tensor.matmul(
                    out=arena[:, off : off + 512],
                    lhsT=w1sb[32 * i : 32 * i + 12,
                              128 * img : 128 * img + 128],
                    rhs=xt[32 * i : 32 * i + 12,
                           512 * (g % 2) : 512 * (g % 2) + 512],
                    start=True,
                    stop=True,
                    tile_position=(32 * i, 0),
                )

            def mm1_main(g, xt):
                # chunks 1..3 -> banks 1-3 of the pair region; the only WAR
                # is exp_a(g-2), long resolved.
                with tc.high_priority(offset=40):
                    for i in range(1, 4):
                        mm1_one(g, xt, i)

            def mm1_late(g, xt):
                # chunk 0 -> bank 0 (the previous p2): WAR on copy(g-2),
                # hidden behind exp_a(g). Priority lands it between mm2(g-1)
                # (which feeds its dependency) and mm2(g).
                with tc.high_priority(offset=14):
                    mm1_one(g, xt, 0)

            def exp_pair(g):
                """exp of pair region g%2 -> e tile [128,2048] f16.
                Two instrs: banks 1-3 (N=1536) then bank 0 (N=512), so the
                bank-0 dependency hides behind the big one."""
                img = g // GPI
                base = 2048 * (g % 2)
                if g in DVE_GRAN:
                    # hybrid: DVE fast-exp for banks 1-3, ACT for bank 0
                    ei = e_pool.tile([128, 2048], I16, name="e")
                    nc.vector.tensor_scalar(
                        out=ei[:, 512:2048],
                        in0=arena[:, base + 512 : base + 2048],
                        scalar1=SCH_S,
                        scalar2=vsb[:, img : img + 1],
                        op0=mybir.AluOpType.mult,
                        op1=mybir.AluOpType.add,
                    )
                    ef = ei.bitcast(F16)
                    nc.scalar.activation(
                        out=ef[:, 0:512],
                        in_=arena[:, base : base + 512],
                        func=mybir.ActivationFunctionType.Exp,
                        bias=bsb[:, img : img + 1],
                        scale=1.0,
                    )
                    return ef
                e = e_pool.tile([128, 2048], F16, name="e")
                for lo, hi in ((512, 2048), (0, 512)):
                    nc.scalar.activation(
                        out=e[:, lo:hi],
                        in_=arena[:, base + lo : base + hi],
                        func=mybir.ActivationFunctionType.Exp,
                        bias=bsb[:, img : img + 1],
                        scale=1.0,
                    )
                return e[:]

            def mm2(g, e):
                img = g // GPI
                base = 2048 * (g % 2)
                p2 = arena[:, base : base + 512]
                for j in range(4):
                    nc.tensor.matmul(
                        out=p2[32 * j : 32 * j + 32, :],
                        lhsT=w2sb[:, 32 * img : 32 * img + 32],
                        rhs=e[:, 512 * j : 512 * j + 512],
                        start=True,
                        stop=True,
                        tile_position=(0, 32 * j),
                    )
                return p2

            def copy_gather(g, yp):
                img, gi = divmod(g, GPI)
                base = 2048 * (g % 2)
                half = 512 * (g % 2)
                nc.vector.tensor_copy(
                    out=yp[:, half : half + 512], in_=arena[:, base : base + 512]
                )
                # gather denominator rows {32j+8u} (stride 8, offset 0)
                src = bass.AP(
                    tensor=yp.tensor,
                    offset=yp.offset + half,
                    ap=[[8 * 1024, 16], [1, 512]],
                )
                nc.sync.dma_start(
                    out=sd[64 * (img % 2) + 16 * (gi // 2) :
                           64 * (img % 2) + 16 * (gi // 2) + 16,
                           1024 * (img // 2) + half :
                           1024 * (img // 2) + half + 512],
                    in_=src,
                )

            def recip(img, h=None):
                """1/sd for an image (or half-image h: pairs 2h..2h+1)."""
                r0, c0 = 64 * (img % 2), 1024 * (img // 2)
                n = 64
                if h is not None:
                    r0, n = r0 + 32 * h, 32
                rr = r[r0 : r0 + n, c0 : c0 + 1024]
                nc.scalar.activation(
                    out=rr,
                    in_=sd[r0 : r0 + n, c0 : c0 + 1024],
                    func=mybir.ActivationFunctionType.Ln,
                )
                nc.scalar.activation(
                    out=rr,
                    in_=rr,
                    func=mybir.ActivationFunctionType.Exp,
                    scale=-1.0,
                )

            def normalize(p, yp):
                img, q = divmod(p, 4)
                rb = rbs[p % 3]
                # rb row 8t+1+c <- r row 64*(img%2)+16q+t, one DMA per c
                src = bass.AP(
                    tensor=r.tensor,
                    offset=r.offset
                    + (64 * (img % 2) + 16 * q) * (512 * IMGS)
                    + 1024 * (img // 2),
                    ap=[[512 * IMGS, 16], [1, 1024]],
                )
                for cc, eng in ((0, nc.gpsimd), (1, nc.gpsimd), (2, nc.sync)):
                    dst = bass.AP(
                        tensor=rb.tensor,
                        offset=rb.offset + (1 + cc) * 1024,
                        ap=[[8 * 1024, 16], [1, 1024]],
                    )
                    eng.dma_start(out=dst, in_=src)
                o = o_pool.tile([128, 1024], F16, name="o")
                nc.vector.tensor_mul(out=o[:], in0=yp[:], in1=rb[:])
                nc.sync.dma_start(out=out[p], in_=o[:])

            yps = {}
            xts = {0: load_x(0), 1: load_x(1)}
            mm1_main(0, xts[0])
            mm1_late(0, xts[0])
            for g in range(NGRAN):
                p = g // 2
                if g % 2 == 0:
                    if p + 2 < NGRAN // 2:
                        xts[p + 2] = load_x(p + 2)
                    yps[p] = y_pool.tile([128, 1024], F16, name="yp")
                if g + 1 < NGRAN:
                    mm1_main(g + 1, xts[(g + 1) // 2])
                e = exp_pair(g)
                mm2(g, e)
                if g + 1 < NGRAN:
                    mm1_late(g + 1, xts[(g + 1) // 2])
                    if g % 2 == 1:
                        xts.pop(p)
                copy_gather(g, yps[p])
                # recip(img) two granules after its last gather so the
                # gather DMA latency never stalls the ACT queue. Image 3
                # splits in halves so pairs 12-13 normalize in-loop.
                if g in (9, 17, 25):
                    recip((g - 9) // 8)
                elif g == 29:
                    recip(3, 0)
                if g >= 9 and g % 2 == 1:
                    normalize((g - 9) // 2, yps.pop((g - 9) // 2))
                if g == 30:
                    normalize(12, yps.pop(12))
                if g == 31:
                    normalize(13, yps.pop(13))
            recip(3, 1)
            for p in range(14, 16):
                normalize(p, yps.pop(p))

    _split_excess_waits(nc)
    return nc


_PROGRAM = None


def _get_program():
    global _PROGRAM
    if _PROGRAM is None:
        _PROGRAM = _build_program()
    return _PROGRAM


# ---------------------------------------------------------------------------
# Host-side prep / decode
# ---------------------------------------------------------------------------


def _prep_core_inputs(images4, palettes4, temperature):
    T = float(temperature)
    # x rows 32i + 3u + c  <- channel c of pixel 512u+n of chunk i
    # chunk i of granule g: block 2*(g%8)+(i>>1), half i&1 of image g//8
    im = images4.reshape(IMGS, GPI, 2, 2, 4, 512, C).transpose(0, 1, 2, 3, 4, 6, 5)
    # [img, gi, bb, h, u, c, n] -> [g, i, 12, 512]
    im = im.reshape(NGRAN, 4, 12, 512)
    xs = np.zeros((NGRAN, 4, 32, 512), np.float16)
    xs[:, :, :12, :] = im
    # pack granule pairs side by side: [16, 128, 1024]
    xs = (xs.reshape(NGRAN // 2, 2, 128, 512)
            .transpose(0, 2, 1, 3)
            .reshape(NGRAN // 2, 128, 1024))
    xs = np.ascontiguousarray(xs)
    w1 = np.zeros((128, 128 * IMGS), np.float16)
    w2 = np.zeros((128, 32 * IMGS), np.float16)
    bias = np.zeros((128, IMGS), np.float32)
    for img in range(IMGS):
        pal = palettes4[img].astype(np.float32)
        pt = ((2.0 / T) * pal.T).astype(np.float16)  # [3, 32]
        pal16 = pal.astype(np.float16)
        bvec = -(pal * pal).sum(axis=1) / T
        for u in range(4):
            for i in range(4):
                w1[32 * i + 3 * u : 32 * i + 3 * u + 3,
                   128 * img + 32 * u : 128 * img + 32 * u + 32] = pt
            bias[32 * u : 32 * u + 32, img] = bvec
            w2[32 * u : 32 * u + 32, 32 * img + 8 * u] = 1.0
            w2[32 * u : 32 * u + 32,
               32 * img + 8 * u + 1 : 32 * img + 8 * u + 4] = pal16
    vbias = (bias * np.float32(SCH_S) + np.float32(SCH_M)).astype(np.float32)
    return {"x": xs, "w1": w1, "w2": w2, "bias": bias, "vbias": vbias}


def _decode_core_output(out_core):
    """out [16, 128(32j+8u+1+c), 1024(pair)] f16 -> [IMGS, NPIX, C] f32."""
    out_core = (out_core.reshape(NGRAN // 2, 128, 2, 512)
                .transpose(0, 2, 1, 3)
                .reshape(NGRAN, 128, 512))
    o = out_core.reshape(IMGS, GPI, 2, 2, 4, 8, 512)[:, :, :, :, :, 1:4, :]
    # [img, gi, bb, h, u, c, n] -> [img, gi, bb, h, u, n, c]
    o = o.astype(np.float32).transpose(0, 1, 2, 3, 4, 6, 5)
    return o.reshape(IMGS, NPIX, C)


# ---------------------------------------------------------------------------
# Entry points
# ---------------------------------------------------------------------------


def run(images, palettes, temperature, trace=False):
    """Returns (output [B,H,W,C] f32, exec_time_ns or None)."""
    from concourse.bass_utils import run_bass_kernel_spmd

    images = np.asarray(images, np.float32)
    palettes = np.asarray(palettes, np.float32)
    nc = _get_program()
    in_maps = [
        _prep_core_inputs(
            images[IMGS * c : IMGS * (c + 1)].reshape(IMGS, NPIX, C),
            palettes[IMGS * c : IMGS * (c + 1)],
            temperature,
        )
        for c in range(N_CORES)
    ]
    res = run_bass_kernel_spmd(nc, in_maps, list(range(N_CORES)), trace=trace)
    outs = [_decode_core_output(res.results[c]["out"]) for c in range(N_CORES)]
    full = np.concatenate(outs, axis=0).reshape(B, H, W, C)
    return full, res.exec_time_ns


def kernel(images, palettes, temperature):
    return run(images, palettes, temperature)[0]


# revision 56
# speedup vs baseline: 1.0407x; 1.0407x over previous
"""TRN2 Bass kernel: differentiable palette quantization (soft VQ).

  weights = softmax_k( -|x - p_k|^2 / T );  out = sum_k weights_k p_k

Data-parallel over 8 NeuronCores (4 images each). Pipeline per core:

  mm1 (PE, 4-way row-tiled):  logits = 2/T * x.p  ->  PSUM f32
  exp (ACT table / DVE Schraudolph split):        ->  e f16 SBUF
  mm2 (PE, 4-way col-tiled, M=32):                ->  PSUM f32
  copy (DVE) -> y f16; denominator gather (DMA) -> sd
  recip via Ln/-Exp (ACT); rb row-broadcast (DMA); o = y*rb (DVE 2x)

Granule = 2 blocks = 8192 px; 32 granules/core; 8 granules per image.
PSUM: 6-bank p1 ring arena + 2x 1-bank p2.

Row layout within each 32-row output group (so every DMA partition
pattern is a single stride with offset % stride == 0): output
s = 8u is the denominator, s = 8u + 1 + c the channels c<3, rest
zero. sd/r row = 16*gi + 4j + u. rb replicates r to all 8 slots via
one stride-0-source DMA; out DMAs the full [128,512] tile.

Self-contained: includes the walrus sync-wait-limit workaround (this
toolchain allows ONE semaphore wait per instruction) and the Tile exit
drain patch.
"""

import sys

sys.path.insert(0, "/opt/trn_rl_repo")

import math

import numpy as np

import concourse.bass as bass
import concourse.tile as tile
from concourse import mybir
from concourse.tile import ScopedClock

B, H, W, C, K = 32, 256, 256, 3, 32
NPIX = H * W
N_CORES = 8
IMGS = B // N_CORES      # 4 images per core
NGRAN = 32               # granules per core (2 blocks each)
GPI = 8                  # granules per image
F32 = mybir.dt.float32
F16 = mybir.dt.float16
I16 = mybir.dt.int16

SCH_S = float(1024.0 / math.log(2.0))   # Schraudolph scale
SCH_SIGMA = 0.06
SCH_M = float(1024.0 * (15.0 - SCH_SIGMA))

# Granules routed to the DVE fast-exp instead of the ACT exp table.
DVE_GRAN = {g for g in range(32) if g % 5 == 2}



# ---------------------------------------------------------------------------
# Toolchain workarounds
# ---------------------------------------------------------------------------

_MAX_WAITS = 1


def _split_excess_waits(nc):
    """This walrus build rejects >1 sync wait per instruction. Move the
    excess onto same-engine NOPs inserted immediately before."""
    for f in nc.m.functions:
        for bb in f.blocks:
            insts = bb.instructions
            if not any(
                i.sync_info is not None and len(i.sync_info.on_wait) > _MAX_WAITS
                for i in insts
            ):
                continue
            new = []
            for inst in insts:
                si = inst.sync_info
                waits = list(si.on_wait) if si is not None else []
                if len(waits) > _MAX_WAITS:
                    extra, keep = waits[:-_MAX_WAITS], waits[-_MAX_WAITS:]
                    for i in range(0, len(extra), _MAX_WAITS):
                        new.append(
                            mybir.InstNoOp(
                                name=nc.get_next_instruction_name(),
                                engine=inst.engine,
                                bass_nofuse=True,
                                sync_info=mybir.SyncInfo(
                                    on_wait=extra[i : i + _MAX_WAITS], on_update=[]
                                ),
                            )
                        )
                    inst.sync_info = mybir.SyncInfo(
                        on_wait=keep, on_update=list(si.on_update)
                    )
                new.append(inst)
            bb.instructions = new


def _patched_drain_and_barrier(self, tick_clock, wait_clock):
    """Tile's exit drain carries one wait per active proc; spread them
    across single-wait NOPs (same walrus limit as above)."""
    nc = self.nc
    probe = nc.sync.nop(nofuse=True, hint="drain_waits")
    wait_clock.add_sem_waits(probe.ins, ScopedClock({None: tick_clock.global_clock}))
    si = probe.ins.sync_info
    waits = list(si.on_wait) if si is not None else []
    updates = list(si.on_update) if si is not None else []
    if len(waits) > 1:
        probe.ins.sync_info = mybir.SyncInfo(on_wait=waits[:1], on_update=updates)
        for i, w in enumerate(waits[1:]):
            extra = nc.sync.nop(nofuse=True, hint=f"drain_waits_{i}")
            extra.ins.sync_info = mybir.SyncInfo(on_wait=[w], on_update=[])
    nc.sync.drain()
    nc.all_engine_barrier()
    assert self.sems is not None
    popped = nc._tile_sem_poison_stack.pop()
    assert popped is self._sem_poison
    nc.clear_and_free_semaphores(list(self.sems.allocated().values()))
    nc.all_engine_barrier()


tile.TileContext._drain_and_barrier = _patched_drain_and_barrier

# ---------------------------------------------------------------------------
# Program builder
# ---------------------------------------------------------------------------


def _build_program():
    nc = bass.Bass()
    NP = NGRAN // 2  # 16 granule-pairs
    x = nc.dram_tensor("x", [NP, 128, 1024], F16, kind="ExternalInput")
    w1 = nc.dram_tensor("w1", [128, 128 * IMGS], F16, kind="ExternalInput")
    w2 = nc.dram_tensor("w2", [128, 32 * IMGS], F16, kind="ExternalInput")
    bias = nc.dram_tensor("bias", [128, IMGS], F32, kind="ExternalInput")
    vbias = nc.dram_tensor("vbias", [128, IMGS], F32, kind="ExternalInput")
    out = nc.dram_tensor("out", [NP, 128, 1024], F16, kind="ExternalOutput")

    with tile.TileContext(nc) as tc:
        with (
            tc.tile_pool(name="singles", bufs=1) as singles,
            tc.tile_pool(name="xt", bufs=4) as xt_pool,
            tc.tile_pool(name="e", bufs=4) as e_pool,
            tc.tile_pool(name="arena", bufs=1, space="PSUM") as arena_pool,
            tc.tile_pool(name="y", bufs=8) as y_pool,
            tc.tile_pool(name="o", bufs=3) as o_pool,
        ):
            w1sb = singles.tile([128, 128 * IMGS], F16)
            w2sb = singles.tile([128, 32 * IMGS], F16)
            bsb = singles.tile([128, IMGS], F32)
            vsb = singles.tile([128, IMGS], F32)
            # sd/r layout: row = 64*(img%2) + 16*q + t  (q = pair-in-image,
            # t = 4j+u), col = 1024*(img//2) + 512*(g%2) + n.
            sd = singles.tile([128, 512 * IMGS], F16)
            r = singles.tile([128, 512 * IMGS], F16)
            rbs = []
            for i in range(3):
                rbt = singles.tile([128, 1024], F16, name=f"rbt{i}")
                nc.vector.memset(rbt[:], 0.0)
                rbs.append(rbt)
            # init loads off the sync queue so the first xt DMAs go first;
            # scratch exp preloads the ACT exp/ln table during the DMAs.
            scratch = singles.tile([128, 1], F16)
            nc.gpsimd.dma_start(out=w1sb[:], in_=w1[:])
            nc.scalar.dma_start(out=w2sb[:], in_=w2[:])
            nc.gpsimd.dma_start(out=bsb[:], in_=bias[:])
            nc.gpsimd.dma_start(out=vsb[:], in_=vbias[:])
            nc.vector.memset(scratch[:], 1.0)
            nc.scalar.activation(
                out=scratch[:], in_=scratch[:],
                func=mybir.ActivationFunctionType.Exp,
            )

            # All 8 PSUM banks: two pair-regions of [128,2048] (4 block-
            # slots). mm1(g) fills pair g%2; exp(g) reads it as one N=2048;
            # mm2(g) then reuses the first bank of the pair as its p2.
            arena = arena_pool.tile([128, 4096], F32)

            def load_x(p):
                xt = xt_pool.tile([128, 1024], F16, name="xt")
                with tc.high_priority(offset=40):
                    nc.sync.dma_start(out=xt[:], in_=x[p])
                return xt

            def mm1_one(g, xt, i):
                img = g // GPI
                off = 2048 * (g % 2) + 512 * i
                nc.tensor.matmul(
                    out=arena[:, off : off + 512],
                    lhsT=w1sb[32 * i : 32 * i + 12,
                              128 * img : 128 * img + 128],
                    rhs=xt[32 * i : 32 * i + 12,
                           512 * (g % 2) : 512 * (g % 2) + 512],
                    start=True,
                    stop=True,
                    tile_position=(32 * i, 0),
                )

            def mm1_main(g, xt):
                # chunks 1..3 -> banks 1-3 of the pair region; the only WAR
                # is exp_a(g-2), long resolved.
                with tc.high_priority(offset=40):
                    for i in range(1, 4):
                        mm1_one(g, xt, i)

            def mm1_late(g, xt):
                # chunk 0 -> bank 0 (the previous p2): WAR on copy(g-2),
                # hidden behind exp_a(g). Priority lands it between mm2(g-1)
                # (which feeds its dependency) and mm2(g).
                with tc.high_priority(offset=14):
                    mm1_one(g, xt, 0)

            def exp_pair(g):
                """exp of pair region g%2 -> e tile [128,2048] f16.
                Two instrs: banks 1-3 (N=1536) then bank 0 (N=512), so the
                bank-0 dependency hides behind the big one."""
                img = g // GPI
                base = 2048 * (g % 2)
                if g in DVE_GRAN:
                    ei = e_pool.tile([128, 2048], I16, name="e")
                    for lo, hi in ((512, 2048), (0, 512)):
                        nc.vector.tensor_scalar(
                            out=ei[:, lo:hi],
                            in0=arena[:, base + lo : base + hi],
                            scalar1=SCH_S,
                            scalar2=vsb[:, img : img + 1],
                            op0=mybir.AluOpType.mult,
                            op1=mybir.AluOpType.add,
                        )
                    return ei.bitcast(F16)
                e = e_pool.tile([128, 2048], F16, name="e")
                for lo, hi in ((512, 2048), (0, 512)):
                    nc.scalar.activation(
                        out=e[:, lo:hi],
                        in_=arena[:, base + lo : base + hi],
                        func=mybir.ActivationFunctionType.Exp,
                        bias=bsb[:, img : img + 1],
                        scale=1.0,
                    )
                return e[:]

            def mm2(g, e):
                img = g // GPI
                base = 2048 * (g % 2)
                p2 = arena[:, base : base + 512]
                for j in range(4):
                    nc.tensor.matmul(
                        out=p2[32 * j : 32 * j + 32, :],
                        lhsT=w2sb[:, 32 * img : 32 * img + 32],
                        rhs=e[:, 512 * j : 512 * j + 512],
                        start=True,
                        stop=True,
                        tile_position=(0, 32 * j),
                    )
                return p2

            def copy_gather(g, yp):
                img, gi = divmod(g, GPI)
                base = 2048 * (g % 2)
                half = 512 * (g % 2)
                nc.vector.tensor_copy(
                    out=yp[:, half : half + 512], in_=arena[:, base : base + 512]
                )
                # gather denominator rows {32j+8u} (stride 8, offset 0)
                src = bass.AP(
                    tensor=yp.tensor,
                    offset=yp.offset + half,
                    ap=[[8 * 1024, 16], [1, 512]],
                )
                nc.sync.dma_start(
                    out=sd[64 * (img % 2) + 16 * (gi // 2) :
                           64 * (img % 2) + 16 * (gi // 2) + 16,
                           1024 * (img // 2) + half :
                           1024 * (img // 2) + half + 512],
                    in_=src,
                )

            def recip(img, h=None):
                """1/sd for an image (or half-image h: pairs 2h..2h+1)."""
                r0, c0 = 64 * (img % 2), 1024 * (img // 2)
                n = 64
                if h is not None:
                    r0, n = r0 + 32 * h, 32
                rr = r[r0 : r0 + n, c0 : c0 + 1024]
                nc.scalar.activation(
                    out=rr,
                    in_=sd[r0 : r0 + n, c0 : c0 + 1024],
                    func=mybir.ActivationFunctionType.Ln,
                )
                nc.scalar.activation(
                    out=rr,
                    in_=rr,
                    func=mybir.ActivationFunctionType.Exp,
                    scale=-1.0,
                )

            def normalize(p, yp):
                img, q = divmod(p, 4)
                rb = rbs[p % 3]
                # rb row 8t+1+c <- r row 64*(img%2)+16q+t, one DMA per c
                src = bass.AP(
                    tensor=r.tensor,
                    offset=r.offset
                    + (64 * (img % 2) + 16 * q) * (512 * IMGS)
                    + 1024 * (img // 2),
                    ap=[[512 * IMGS, 16], [1, 1024]],
                )
                for cc, eng in ((0, nc.gpsimd), (1, nc.gpsimd), (2, nc.sync)):
                    dst = bass.AP(
                        tensor=rb.tensor,
                        offset=rb.offset + (1 + cc) * 1024,
                        ap=[[8 * 1024, 16], [1, 1024]],
                    )
                    eng.dma_start(out=dst, in_=src)
                o = o_pool.tile([128, 1024], F16, name="o")
                nc.vector.tensor_mul(out=o[:], in0=yp[:], in1=rb[:])
                nc.sync.dma_start(out=out[p], in_=o[:])

            yps = {}
            xts = {0: load_x(0), 1: load_x(1)}
            mm1_main(0, xts[0])
            mm1_late(0, xts[0])
            for g in range(NGRAN):
                p = g // 2
                if g % 2 == 0:
                    if p + 2 < NGRAN // 2:
                        xts[p + 2] = load_x(p + 2)
                    yps[p] = y_pool.tile([128, 1024], F16, name="yp")
                if g + 1 < NGRAN:
                    mm1_main(g + 1, xts[(g + 1) // 2])
                e = exp_pair(g)
                mm2(g, e)
                if g + 1 < NGRAN:
                    mm1_late(g + 1, xts[(g + 1) // 2])
                    if g % 2 == 1:
                        xts.pop(p)
                copy_gather(g, yps[p])
                # recip(img) two granules after its last gather so the
                # gather DMA latency never stalls the ACT queue. Image 3
                # splits in halves so pairs 12-13 normalize in-loop.
                if g in (9, 17, 25):
                    recip((g - 9) // 8)
                elif g == 29:
                    recip(3, 0)
                if g >= 9 and g % 2 == 1:
                    normalize((g - 9) // 2, yps.pop((g - 9) // 2))
                if g == 30:
                    normalize(12, yps.pop(12))
                if g == 31:
                    normalize(13, yps.pop(13))
            recip(3, 1)
            for p in range(14, 16):
                normalize(p, yps.pop(p))

    _split_excess_waits(nc)
    return nc


_PROGRAM = None


def _get_program():
    global _PROGRAM
    if _PROGRAM is None:
        _PROGRAM = _build_program()
    return _PROGRAM


# ---------------------------------------------------------------------------
# Host-side prep / decode
# ---------------------------------------------------------------------------


def _prep_core_inputs(images4, palettes4, temperature):
    T = float(temperature)
    # x rows 32i + 3u + c  <- channel c of pixel 512u+n of chunk i
    # chunk i of granule g: block 2*(g%8)+(i>>1), half i&1 of image g//8
    im = images4.reshape(IMGS, GPI, 2, 2, 4, 512, C).transpose(0, 1, 2, 3, 4, 6, 5)
    # [img, gi, bb, h, u, c, n] -> [g, i, 12, 512]
    im = im.reshape(NGRAN, 4, 12, 512)
    xs = np.zeros((NGRAN, 4, 32, 512), np.float16)
    xs[:, :, :12, :] = im
    # pack granule pairs side by side: [16, 128, 1024]
    xs = (xs.reshape(NGRAN // 2, 2, 128, 512)
            .transpose(0, 2, 1, 3)
            .reshape(NGRAN // 2, 128, 1024))
    xs = np.ascontiguousarray(xs)
    w1 = np.zeros((128, 128 * IMGS), np.float16)
    w2 = np.zeros((128, 32 * IMGS), np.float16)
    bias = np.zeros((128, IMGS), np.float32)
    for img in range(IMGS):
        pal = palettes4[img].astype(np.float32)
        pt = ((2.0 / T) * pal.T).astype(np.float16)  # [3, 32]
        pal16 = pal.astype(np.float16)
        bvec = -(pal * pal).sum(axis=1) / T
        for u in range(4):
            for i in range(4):
                w1[32 * i + 3 * u : 32 * i + 3 * u + 3,
                   128 * img + 32 * u : 128 * img + 32 * u + 32] = pt
            bias[32 * u : 32 * u + 32, img] = bvec
            w2[32 * u : 32 * u + 32, 32 * img + 8 * u] = 1.0
            w2[32 * u : 32 * u + 32,
               32 * img + 8 * u + 1 : 32 * img + 8 * u + 4] = pal16
    vbias = (bias * np.float32(SCH_S) + np.float32(SCH_M)).astype(np.float32)
    return {"x": xs, "w1": w1, "w2": w2, "bias": bias, "vbias": vbias}


def _decode_core_output(out_core):
    """out [16, 128(32j+8u+1+c), 1024(pair)] f16 -> [IMGS, NPIX, C] f32."""
    out_core = (out_core.reshape(NGRAN // 2, 128, 2, 512)
                .transpose(0, 2, 1, 3)
                .reshape(NGRAN, 128, 512))
    o = out_core.reshape(IMGS, GPI, 2, 2, 4, 8, 512)[:, :, :, :, :, 1:4, :]
    # [img, gi, bb, h, u, c, n] -> [img, gi, bb, h, u, n, c]
    o = o.astype(np.float32).transpose(0, 1, 2, 3, 4, 6, 5)
    return o.reshape(IMGS, NPIX, C)


# ---------------------------------------------------------------------------
# Entry points
# ---------------------------------------------------------------------------


def run(images, palettes, temperature, trace=False):
    """Returns (output [B,H,W,C] f32, exec_time_ns or None)."""
    from concourse.bass_utils import run_bass_kernel_spmd

    images = np.asarray(images, np.float32)
    palettes = np.asarray(palettes, np.float32)
    nc = _get_program()
    in_maps = [
        _prep_core_inputs(
            images[IMGS * c : IMGS * (c + 1)].reshape(IMGS, NPIX, C),
            palettes[IMGS * c : IMGS * (c + 1)],
            temperature,
        )
        for c in range(N_CORES)
    ]
    res = run_bass_kernel_spmd(nc, in_maps, list(range(N_CORES)), trace=trace)
    outs = [_decode_core_output(res.results[c]["out"]) for c in range(N_CORES)]
    full = np.concatenate(outs, axis=0).reshape(B, H, W, C)
    return full, res.exec_time_ns


def kernel(images, palettes, temperature):
    return run(images, palettes, temperature)[0]
